# revision 20
# baseline (speedup 1.0000x reference)
"""Trainium2 Bass kernel for nn_Attention (Bahdanau-style additive attention).

Reference computation:
    enc = encoder_outputs.transpose(1, 0, 2)            # [B, S, 2H]
    e_proj = enc @ w_e.T                                # [B, S, H]
    energy = tanh(h_proj[:, None, :] + e_proj + b)      # [B, S, H]
    att = energy @ v_w                                  # [B, S]
    out = softmax(att, axis=1)

Sharding: data-parallel over batch, 4 batch rows per core on 8 cores.

Per-core pipeline (heavy matmul in fp8-e4m3 DoubleRow on the PE: K=256
per instruction = 2x bf16 MACs/cycle; measured 216 ns per 128x512
matmul, the 512-column streaming floor):
  - encoder slice is pre-transposed + fp8-quantized on the host into
    [sg, p(e%128), c(e//256), i, s] tiles so each (b, s-group of 512)
    is one contiguous-per-partition DMA
  - weights w_e are host-quantized to fp8 at scale 512 (escapes the
    e4m3 denormal range), laid out as the DoubleRow moving operand
  - main matmul per (b, s-tile of 128): psum[s(128), h(512)] banks (2
    h-groups) accumulate over 8 e-pair chunks; stationary enc chunk
    shared by the 2 h-group matmuls so LDWEIGHTS stays hidden
  - per-h-group pipelined epilogue: DVE scalar_tensor_tensor fuses the
    1/512 descale with the (h_proj + bias) add; ACT applies tanh; the
    v_w dot runs as scalar_tensor_tensor with accum_out (NOTE:
    InstTensorTensorReduce crashes the device - NRT status 101 - in
    every dtype variant; stt+accum_out is the working fusion), halves
    summed into the logit column
  - PE (HAM clock-gate) + ACT (tanh LUT) warmed up with dummy ops on
    DMA-loaded zeros during the initial DMA ramp; first s-group and
    per-chunk weight DMAs spread across queues so real matmuls start
    ~11us in; the last batch row's logit writeback overlaps its tail

Accuracy: fp8 quantization noise on the logits is reduced ~3.5x by a
host-side first-order compensation: the correlated component of the
logit error, sum_h v_h tanh'(pre) (d_enc @ w + enc @ d_w), is estimated
with a mean-field tanh' (Gauss-Hermite over the per-h Gaussian e_proj
distribution) and subtracted from the logits before the host softmax.
h_proj and the final softmax over [32, 2048] are tiny and run on the
host in fp32. Measured: rel err 5.5e-3 (gate 2e-2), HW exec ~247us
(bf16 baseline: 492us; absolute fp8-DR roofline incl. fixed ramp and
teardown: ~243us).
"""

import sys

try:
    import concourse.bass as bass  # noqa: F401
except ImportError:
    sys.path.insert(0, "/opt/trn_rl_repo")

import numpy as np
import ml_dtypes

import concourse.bacc as bacc
import concourse.mybir as mybir
import concourse.tile as tile
from concourse.bass_utils import run_bass_kernel_spmd

HID = 1024
BATCH = 32
SRC_LEN = 2048

N_CORES = 8
B_LOC = BATCH // N_CORES      # 4
E = 2 * HID                   # 2048
N_EP = E // 256               # 8 e-pair chunks (256 contraction each)
SG = 512                      # s per enc DMA tile
N_SG = SRC_LEN // SG          # 4
N_ST = SRC_LEN // 128         # 16 s-tiles per batch row
HG = 512                      # h per psum bank
N_HG = HID // HG              # 2
W_SCALE = 512.0

f32 = mybir.dt.float32
fp16 = mybir.dt.float16
fp8 = mybir.dt.float8e4
DR = mybir.MatmulPerfMode.DoubleRow

_NC_CACHE = {}


def _build():
    nc = bacc.Bacc(
        "TRN2", target_bir_lowering=False, debug=False, num_devices=N_CORES
    )
    enc = nc.declare_dram_parameter(
        "enc", [B_LOC, N_SG, 128, N_EP, 2, SG], fp8, isOutput=False
    )
    wq = nc.declare_dram_parameter("wq", [N_EP, 128, 2, HID], fp8, isOutput=False)
    cbb = nc.declare_dram_parameter("cbb", [B_LOC, 128, HID], f32, isOutput=False)
    vb = nc.declare_dram_parameter("vb", [128, HID], fp16, isOutput=False)
    zz = nc.declare_dram_parameter("zz", [128, 2, 128 + HG], fp8, isOutput=False)
    # [b, p, st]: logit(b, st*128 + p)
    att = nc.declare_dram_parameter("att", [B_LOC, 128, N_ST], f32, isOutput=True)

    with tile.TileContext(nc) as tc:
        with (
            tc.tile_pool(name="const", bufs=1) as const_pool,
            tc.tile_pool(name="cbbp", bufs=2) as cbb_pool,
            tc.tile_pool(name="encp", bufs=6) as enc_pool,
            tc.tile_pool(name="prep", bufs=4) as pre_pool,
            tc.tile_pool(name="tep", bufs=5) as te_pool,
            tc.tile_pool(name="junkp", bufs=2) as junk_pool,
            tc.tile_pool(name="accp", bufs=3) as acc_pool,
            tc.tile_pool(name="attsb", bufs=1) as att_pool,
            tc.tile_pool(name="psum", bufs=6, space="PSUM") as psum_pool,
            tc.tile_pool(name="psumw", bufs=1, space="PSUM") as psumw_pool,
        ):
            wq_sb = const_pool.tile([128, N_EP, 2, HID], fp8)
            vb_sb = const_pool.tile([128, HID], fp16)
            att_sb = att_pool.tile([128, B_LOC * N_ST], f32)

            cbb_sbs = [None] * B_LOC

            def load_cbb(b):
                t = cbb_pool.tile([128, HID], f32, tag="cbb", name=f"cbb_{b}")
                nc.sync.dma_start(t[:], cbb[b])
                cbb_sbs[b] = t

            def load_enc(b, sg, split=1):
                t = enc_pool.tile(
                    [128, N_EP, 2, SG], fp8, tag="enc", name=f"enc_{b}_{sg}"
                )
                step = N_EP // split
                for k in range(split):
                    nc.sync.dma_start(
                        t[:, k * step:(k + 1) * step],
                        enc[b, sg, :, k * step:(k + 1) * step],
                    )
                return t

            # ---- startup DMAs: warmup zeros, then the first s-group and wq
            # chunks spread across queues so the first matmuls issue early ----
            z = const_pool.tile([128, 2, 128 + HG], fp8)
            nc.sync.dma_start(z[:], zz[:])
            encs = [load_enc(0, 0, split=4)]
            nc.sync.dma_start(wq_sb[:, 0], wq[0])
            nc.sync.dma_start(wq_sb[:, 1], wq[1])
            for c in range(2, N_EP):
                nc.sync.dma_start(wq_sb[:, c], wq[c])
            load_cbb(0)
            nc.sync.dma_start(vb_sb[:], vb[:])
            encs.append(load_enc(0, 1, split=2))
            for sg in range(2, N_SG):
                encs.append(load_enc(0, sg))

            # ---- PE (HAM) + ACT (tanh LUT) warmup during the DMA ramp ----
            warm_ps = psumw_pool.tile([128, HG], f32)
            for k in range(2):
                nc.tensor.matmul(
                    warm_ps[:], lhsT=z[:, :, 0:128], rhs=z[:, :, 128:128 + HG],
                    start=True, stop=True, perf_mode=DR,
                )
            warm_t = const_pool.tile([128, 1], f32)
            nc.scalar.activation(
                warm_t[:], z[:, 0, 0:1], mybir.ActivationFunctionType.Tanh
            )

            def do_tile(b, st):
                sg, j = st // N_SG, st % N_SG
                ps = [
                    psum_pool.tile([128, HG], f32, tag="ps", name=f"ps_{b}_{st}_{g}")
                    for g in range(N_HG)
                ]
                for c in range(N_EP):
                    for hg in range(N_HG):
                        nc.tensor.matmul(
                            ps[hg][:],
                            lhsT=encs[sg][:, c, :, j * 128:(j + 1) * 128],
                            rhs=wq_sb[:, c, :, hg * HG:(hg + 1) * HG],
                            start=(c == 0),
                            stop=(c == N_EP - 1),
                            perf_mode=DR,
                        )
                # pipelined half-width epilogue: each h-group's descale+bias
                # (DVE), tanh (ACT) and v-dot partial (DVE, via accum_out --
                # tensor_tensor_reduce crashes the device) proceed as soon as
                # its psum bank completes; halves summed into the logit column
                pre = pre_pool.tile([128, HID], fp16, tag="pre", name=f"pre_{b}_{st}")
                te = te_pool.tile([128, HID], fp16, tag="te", name=f"te_{b}_{st}")
                junk = junk_pool.tile(
                    [128, HID], fp16, tag="junk", name=f"junk_{b}_{st}"
                )
                acc = acc_pool.tile([128, 2], f32, tag="acc", name=f"acc_{b}_{st}")
                for hg in range(N_HG):
                    sl = slice(hg * HG, (hg + 1) * HG)
                    nc.vector.scalar_tensor_tensor(
                        out=pre[:, sl],
                        in0=ps[hg][:],
                        scalar=1.0 / W_SCALE,
                        in1=cbb_sbs[b][:, sl],
                        op0=mybir.AluOpType.mult,
                        op1=mybir.AluOpType.add,
                    )
                    nc.scalar.activation(
                        te[:, sl], pre[:, sl], mybir.ActivationFunctionType.Tanh
                    )
                    nc.vector.scalar_tensor_tensor(
                        out=junk[:, sl],
                        in0=te[:, sl],
                        scalar=1.0,
                        in1=vb_sb[:, sl],
                        op0=mybir.AluOpType.bypass,
                        op1=mybir.AluOpType.mult,
                        accum_out=acc[:, hg:hg + 1],
                    )
                col = b * N_ST + st
                nc.vector.tensor_add(
                    out=att_sb[:, col:col + 1],
                    in0=acc[:, 0:1],
                    in1=acc[:, 1:2],
                )

            for b in range(B_LOC):
                last = b == B_LOC - 1
                encs_next = []
                for st in range(N_ST):
                    if not last:
                        if st == 1:
                            load_cbb(b + 1)
                        elif st in (2, 5, 8, 11):
                            encs_next.append(load_enc(b + 1, len(encs_next)))
                    do_tile(b, st)
                    if last and st == N_ST - 2:
                        # overlap most of the final logit writeback with the
                        # last tile's compute
                        nc.sync.dma_start(
                            att[b, :, 0:N_ST - 1],
                            att_sb[:, b * N_ST:b * N_ST + N_ST - 1],
                        )
                if last:
                    nc.sync.dma_start(
                        att[b, :, N_ST - 1:N_ST],
                        att_sb[:, b * N_ST + N_ST - 1:(b + 1) * N_ST],
                    )
                else:
                    nc.sync.dma_start(att[b], att_sb[:, b * N_ST:(b + 1) * N_ST])
                encs = encs_next
    nc.compile()
    return nc


def _get_nc():
    if "nc" not in _NC_CACHE:
        _NC_CACHE["nc"] = _build()
    return _NC_CACHE["nc"]


def kernel(hidden, encoder_outputs, attn_w, attn_b, v_w, _trace=False):
    hidden = np.asarray(hidden, dtype=np.float32)
    encoder_outputs = np.asarray(encoder_outputs, dtype=np.float32)
    attn_w = np.asarray(attn_w, dtype=np.float32)
    attn_b = np.asarray(attn_b, dtype=np.float32)
    v_w = np.asarray(v_w, dtype=np.float32)

    c_b = hidden @ attn_w[:, :HID].T + attn_b          # [B, H] fp32
    w_e = attn_w[:, HID:]                              # [H, E]

    wq_f = (w_e * W_SCALE).astype(ml_dtypes.float8_e4m3)
    w8 = wq_f.astype(np.float32) / W_SCALE             # effective weights
    d_w = w8 - w_e                                     # [H, E]
    # device layout [c, p, i, h]: wq[c,p,i,h] = q(512*w_e[h, 256c+128i+p])
    wq_dev = np.ascontiguousarray(
        wq_f.T.reshape(N_EP, 2, 128, HID).transpose(0, 2, 1, 3)
    )
    vb_dev = np.ascontiguousarray(
        np.broadcast_to(v_w[None, :], (128, HID))
    ).astype(np.float16)

    # mean-field tanh': pre[b,s,h] ~ N(c_b[b,h], sigma_h^2), sigma_h = ||w_e[h,:]||
    sig = np.linalg.norm(w_e, axis=1)                  # [H]
    nodes, wts = np.polynomial.hermite_e.hermegauss(21)
    zz = c_b[None, :, :] + sig[None, None, :] * nodes[:, None, None]
    mbar = (wts[:, None, None] * (1.0 - np.tanh(zz) ** 2)).sum(0) / wts.sum()
    vm = v_w[None, :] * mbar                           # [B, H]
    T_w = vm @ d_w                                     # [B, E]
    g_e = vm @ w_e                                     # [B, E]

    nc = _get_nc()
    in_maps = []
    corr = np.empty((BATCH, SRC_LEN), dtype=np.float32)
    for core in range(N_CORES):
        b0 = core * B_LOC
        enc_dev = np.empty((B_LOC, N_SG, 128, N_EP, 2, SG), dtype=ml_dtypes.float8_e4m3)
        for b in range(B_LOC):
            eb = np.ascontiguousarray(encoder_outputs[:, b0 + b, :])  # [S, E] f32
            q = eb.astype(ml_dtypes.float8_e4m3)
            qf = q.astype(np.float32)
            # first-order compensation of the correlated quantization error
            corr[b0 + b] = eb @ T_w[b0 + b] + (qf - eb) @ g_e[b0 + b]
            # [S,E] -> [sg, p, c, i, s]
            enc_dev[b] = (
                q.reshape(N_SG, SG, N_EP, 2, 128).transpose(0, 4, 2, 3, 1)
            )
        cbb_dev = np.ascontiguousarray(
            np.broadcast_to(c_b[b0:b0 + B_LOC, None, :], (B_LOC, 128, HID))
        ).astype(np.float32)
        in_maps.append(
            {"enc": np.ascontiguousarray(enc_dev), "wq": wq_dev,
             "cbb": cbb_dev, "vb": vb_dev,
             "zz": np.zeros((128, 2, 128 + HG), dtype=ml_dtypes.float8_e4m3)}
        )

    res = run_bass_kernel_spmd(
        nc, in_maps, core_ids=list(range(N_CORES)), trace=_trace
    )
    if _trace:
        _NC_CACHE["last_result"] = res

    att = np.concatenate(
        [
            res.results[c]["att"].transpose(0, 2, 1).reshape(B_LOC, SRC_LEN)
            for c in range(N_CORES)
        ],
        axis=0,
    )  # [B, S] logits
    att = att - corr

    m = att.max(axis=1, keepdims=True)
    e = np.exp(att - m)
    out = e / e.sum(axis=1, keepdims=True)
    return out.astype(np.float32)


# revision 21
# speedup vs baseline: 1.1912x; 1.1912x over previous
"""Trainium2 Bass kernel for nn_Attention (Bahdanau-style additive attention).

Reference computation:
    enc = encoder_outputs.transpose(1, 0, 2)            # [B, S, 2H]
    e_proj = enc @ w_e.T                                # [B, S, H]
    energy = tanh(h_proj[:, None, :] + e_proj + b)      # [B, S, H]
    att = energy @ v_w                                  # [B, S]
    out = softmax(att, axis=1)

Sharding: data-parallel over batch, 4 batch rows per core on 8 cores.

Per-core pipeline (heavy matmul in fp8-e4m3 DoubleRow on the PE: K=256
per instruction = 2x bf16 MACs/cycle; measured 216 ns per 128x512
matmul, the 512-column streaming floor):
  - encoder slice is pre-transposed + fp8-quantized on the host into
    [sg, p(e%128), c(e//256), i, s] tiles so each (b, s-group of 512)
    is one contiguous-per-partition DMA
  - weights w_e are host-quantized to fp8 at scale 512 (escapes the
    e4m3 denormal range), laid out as the DoubleRow moving operand
  - main matmul per (b, s-tile of 128): psum[s(128), h(512)] banks (2
    h-groups) accumulate over 8 e-pair chunks; stationary enc chunk
    shared by the 2 h-group matmuls so LDWEIGHTS stays hidden
  - per-h-group pipelined epilogue: DVE scalar_tensor_tensor fuses the
    1/512 descale with the (h_proj + bias) add; ACT applies tanh; the
    v_w dot runs as scalar_tensor_tensor with accum_out (NOTE:
    InstTensorTensorReduce crashes the device - NRT status 101 - in
    every dtype variant; stt+accum_out is the working fusion), halves
    summed into the logit column
  - PE (HAM clock-gate) + ACT (tanh LUT) warmed up with dummy ops on
    DMA-loaded zeros during the initial DMA ramp; first s-group and
    per-chunk weight DMAs spread across queues so real matmuls start
    ~11us in; the last batch row's logit writeback overlaps its tail

Accuracy: fp8 quantization noise on the logits is reduced ~3.5x by a
host-side first-order compensation: the correlated component of the
logit error, sum_h v_h tanh'(pre) (d_enc @ w + enc @ d_w), is estimated
with a mean-field tanh' (Gauss-Hermite over the per-h Gaussian e_proj
distribution) and subtracted from the logits before the host softmax.
h_proj and the final softmax over [32, 2048] are tiny and run on the
host in fp32. Measured: rel err 5.5e-3 (gate 2e-2), HW exec ~247us
(bf16 baseline: 492us; absolute fp8-DR roofline incl. fixed ramp and
teardown: ~243us).
"""

import sys

try:
    import concourse.bass as bass  # noqa: F401
except ImportError:
    sys.path.insert(0, "/opt/trn_rl_repo")

import numpy as np
import ml_dtypes

import concourse.bacc as bacc
import concourse.mybir as mybir
import concourse.tile as tile
from concourse.bass_utils import run_bass_kernel_spmd

HID = 1024
BATCH = 32
SRC_LEN = 2048

N_CORES = 8
B_LOC = BATCH // N_CORES      # 4
E = 2 * HID                   # 2048
N_EP = E // 256               # 8 e-pair chunks (256 contraction each)
SG = 512                      # s per enc DMA tile
N_SG = SRC_LEN // SG          # 4
N_ST = SRC_LEN // 128         # 16 s-tiles per batch row
HG = 512                      # h per psum bank
N_HG = HID // HG              # 2
W_SCALE = 512.0

f32 = mybir.dt.float32
fp16 = mybir.dt.float16
fp8 = mybir.dt.float8e4
DR = mybir.MatmulPerfMode.DoubleRow

_NC_CACHE = {}


def _build():
    nc = bacc.Bacc(
        "TRN2", target_bir_lowering=False, debug=False, num_devices=N_CORES
    )
    enc = nc.declare_dram_parameter(
        "enc", [B_LOC, N_SG, 128, N_EP, 2, SG], fp8, isOutput=False
    )
    wq = nc.declare_dram_parameter("wq", [N_EP, 128, 2, HID], fp8, isOutput=False)
    cbb = nc.declare_dram_parameter("cbb", [B_LOC, 128, HID], f32, isOutput=False)
    vb = nc.declare_dram_parameter("vb", [128, HID], fp16, isOutput=False)
    zz = nc.declare_dram_parameter("zz", [128, 2, 128 + HG], fp8, isOutput=False)
    # [b, p, st]: logit(b, st*128 + p)
    att = nc.declare_dram_parameter("att", [B_LOC, 128, N_ST], f32, isOutput=True)

    with tile.TileContext(nc) as tc:
        with (
            tc.tile_pool(name="const", bufs=1) as const_pool,
            tc.tile_pool(name="cbbp", bufs=2) as cbb_pool,
            tc.tile_pool(name="encp", bufs=6) as enc_pool,
            tc.tile_pool(name="prep", bufs=4) as pre_pool,
            tc.tile_pool(name="tep", bufs=5) as te_pool,
            tc.tile_pool(name="junkp", bufs=2) as junk_pool,
            tc.tile_pool(name="accp", bufs=3) as acc_pool,
            tc.tile_pool(name="attsb", bufs=1) as att_pool,
            tc.tile_pool(name="psum", bufs=6, space="PSUM") as psum_pool,
            tc.tile_pool(name="psumw", bufs=1, space="PSUM") as psumw_pool,
        ):
            wq_sb = const_pool.tile([128, N_EP, 2, HID], fp8)
            vb_sb = const_pool.tile([128, HID], fp16)
            att_sb = att_pool.tile([128, B_LOC * N_ST], f32)

            cbb_sbs = [None] * B_LOC

            def load_cbb(b):
                t = cbb_pool.tile([128, HID], f32, tag="cbb", name=f"cbb_{b}")
                nc.sync.dma_start(t[:], cbb[b])
                cbb_sbs[b] = t

            def load_enc(b, sg, split=1):
                t = enc_pool.tile(
                    [128, N_EP, 2, SG], fp8, tag="enc", name=f"enc_{b}_{sg}"
                )
                step = N_EP // split
                for k in range(split):
                    nc.sync.dma_start(
                        t[:, k * step:(k + 1) * step],
                        enc[b, sg, :, k * step:(k + 1) * step],
                    )
                return t

            # ---- startup DMAs: warmup zeros, then the first s-group and wq
            # chunks spread across queues so the first matmuls issue early ----
            z = const_pool.tile([128, 2, 128 + HG], fp8)
            nc.sync.dma_start(z[:], zz[:])
            encs = [load_enc(0, 0, split=4)]
            nc.sync.dma_start(wq_sb[:, 0], wq[0])
            nc.sync.dma_start(wq_sb[:, 1], wq[1])
            for c in range(2, N_EP):
                nc.sync.dma_start(wq_sb[:, c], wq[c])
            load_cbb(0)
            nc.sync.dma_start(vb_sb[:], vb[:])
            encs.append(load_enc(0, 1, split=2))
            for sg in range(2, N_SG):
                encs.append(load_enc(0, sg))

            # ---- PE (HAM) + ACT (tanh LUT) warmup during the DMA ramp ----
            warm_ps = psumw_pool.tile([128, HG], f32)
            for k in range(8):
                nc.tensor.matmul(
                    warm_ps[:], lhsT=z[:, :, 0:128], rhs=z[:, :, 128:128 + HG],
                    start=True, stop=True, perf_mode=DR,
                )
            warm_t = const_pool.tile([128, 1], f32)
            nc.scalar.activation(
                warm_t[:], z[:, 0, 0:1], mybir.ActivationFunctionType.Tanh
            )

            def do_tile(b, st):
                sg, j = st // N_SG, st % N_SG
                ps = [
                    psum_pool.tile([128, HG], f32, tag="ps", name=f"ps_{b}_{st}_{g}")
                    for g in range(N_HG)
                ]
                for c in range(N_EP):
                    for hg in range(N_HG):
                        nc.tensor.matmul(
                            ps[hg][:],
                            lhsT=encs[sg][:, c, :, j * 128:(j + 1) * 128],
                            rhs=wq_sb[:, c, :, hg * HG:(hg + 1) * HG],
                            start=(c == 0),
                            stop=(c == N_EP - 1),
                            perf_mode=DR,
                        )
                # pipelined half-width epilogue: each h-group's descale+bias
                # (DVE), tanh (ACT) and v-dot partial (DVE, via accum_out --
                # tensor_tensor_reduce crashes the device) proceed as soon as
                # its psum bank completes; halves summed into the logit column
                pre = pre_pool.tile([128, HID], fp16, tag="pre", name=f"pre_{b}_{st}")
                te = te_pool.tile([128, HID], fp16, tag="te", name=f"te_{b}_{st}")
                junk = junk_pool.tile(
                    [128, HID], fp16, tag="junk", name=f"junk_{b}_{st}"
                )
                acc = acc_pool.tile([128, 2], f32, tag="acc", name=f"acc_{b}_{st}")
                for hg in range(N_HG):
                    sl = slice(hg * HG, (hg + 1) * HG)
                    nc.vector.scalar_tensor_tensor(
                        out=pre[:, sl],
                        in0=ps[hg][:],
                        scalar=1.0 / W_SCALE,
                        in1=cbb_sbs[b][:, sl],
                        op0=mybir.AluOpType.mult,
                        op1=mybir.AluOpType.add,
                    )
                    nc.scalar.activation(
                        te[:, sl], pre[:, sl], mybir.ActivationFunctionType.Tanh
                    )
                    nc.vector.scalar_tensor_tensor(
                        out=junk[:, sl],
                        in0=te[:, sl],
                        scalar=1.0,
                        in1=vb_sb[:, sl],
                        op0=mybir.AluOpType.bypass,
                        op1=mybir.AluOpType.mult,
                        accum_out=acc[:, hg:hg + 1],
                    )
                col = b * N_ST + st
                nc.vector.tensor_add(
                    out=att_sb[:, col:col + 1],
                    in0=acc[:, 0:1],
                    in1=acc[:, 1:2],
                )

            for b in range(B_LOC):
                last = b == B_LOC - 1
                encs_next = []
                for st in range(N_ST):
                    if not last:
                        if st == 1:
                            load_cbb(b + 1)
                        elif st in (2, 5, 8, 11):
                            encs_next.append(load_enc(b + 1, len(encs_next)))
                    do_tile(b, st)
                    if last and st == N_ST - 2:
                        # overlap most of the final logit writeback with the
                        # last tile's compute
                        nc.sync.dma_start(
                            att[b, :, 0:N_ST - 1],
                            att_sb[:, b * N_ST:b * N_ST + N_ST - 1],
                        )
                if last:
                    nc.sync.dma_start(
                        att[b, :, N_ST - 1:N_ST],
                        att_sb[:, b * N_ST + N_ST - 1:(b + 1) * N_ST],
                    )
                else:
                    nc.sync.dma_start(att[b], att_sb[:, b * N_ST:(b + 1) * N_ST])
                encs = encs_next
    nc.compile()
    return nc


def _get_nc():
    if "nc" not in _NC_CACHE:
        _NC_CACHE["nc"] = _build()
    return _NC_CACHE["nc"]


def kernel(hidden, encoder_outputs, attn_w, attn_b, v_w, _trace=False):
    hidden = np.asarray(hidden, dtype=np.float32)
    encoder_outputs = np.asarray(encoder_outputs, dtype=np.float32)
    attn_w = np.asarray(attn_w, dtype=np.float32)
    attn_b = np.asarray(attn_b, dtype=np.float32)
    v_w = np.asarray(v_w, dtype=np.float32)

    c_b = hidden @ attn_w[:, :HID].T + attn_b          # [B, H] fp32
    w_e = attn_w[:, HID:]                              # [H, E]

    wq_f = (w_e * W_SCALE).astype(ml_dtypes.float8_e4m3)
    w8 = wq_f.astype(np.float32) / W_SCALE             # effective weights
    d_w = w8 - w_e                                     # [H, E]
    # device layout [c, p, i, h]: wq[c,p,i,h] = q(512*w_e[h, 256c+128i+p])
    wq_dev = np.ascontiguousarray(
        wq_f.T.reshape(N_EP, 2, 128, HID).transpose(0, 2, 1, 3)
    )
    vb_dev = np.ascontiguousarray(
        np.broadcast_to(v_w[None, :], (128, HID))
    ).astype(np.float16)

    # mean-field tanh': pre[b,s,h] ~ N(c_b[b,h], sigma_h^2), sigma_h = ||w_e[h,:]||
    sig = np.linalg.norm(w_e, axis=1)                  # [H]
    nodes, wts = np.polynomial.hermite_e.hermegauss(21)
    zz = c_b[None, :, :] + sig[None, None, :] * nodes[:, None, None]
    mbar = (wts[:, None, None] * (1.0 - np.tanh(zz) ** 2)).sum(0) / wts.sum()
    vm = v_w[None, :] * mbar                           # [B, H]
    T_w = vm @ d_w                                     # [B, E]
    g_e = vm @ w_e                                     # [B, E]

    nc = _get_nc()
    in_maps = []
    corr = np.empty((BATCH, SRC_LEN), dtype=np.float32)
    for core in range(N_CORES):
        b0 = core * B_LOC
        enc_dev = np.empty((B_LOC, N_SG, 128, N_EP, 2, SG), dtype=ml_dtypes.float8_e4m3)
        for b in range(B_LOC):
            eb = np.ascontiguousarray(encoder_outputs[:, b0 + b, :])  # [S, E] f32
            q = eb.astype(ml_dtypes.float8_e4m3)
            qf = q.astype(np.float32)
            # first-order compensation of the correlated quantization error
            corr[b0 + b] = eb @ T_w[b0 + b] + (qf - eb) @ g_e[b0 + b]
            # [S,E] -> [sg, p, c, i, s]
            enc_dev[b] = (
                q.reshape(N_SG, SG, N_EP, 2, 128).transpose(0, 4, 2, 3, 1)
            )
        cbb_dev = np.ascontiguousarray(
            np.broadcast_to(c_b[b0:b0 + B_LOC, None, :], (B_LOC, 128, HID))
        ).astype(np.float32)
        in_maps.append(
            {"enc": np.ascontiguousarray(enc_dev), "wq": wq_dev,
             "cbb": cbb_dev, "vb": vb_dev,
             "zz": np.zeros((128, 2, 128 + HG), dtype=ml_dtypes.float8_e4m3)}
        )

    res = run_bass_kernel_spmd(
        nc, in_maps, core_ids=list(range(N_CORES)), trace=_trace
    )
    if _trace:
        _NC_CACHE["last_result"] = res

    att = np.concatenate(
        [
            res.results[c]["att"].transpose(0, 2, 1).reshape(B_LOC, SRC_LEN)
            for c in range(N_CORES)
        ],
        axis=0,
    )  # [B, S] logits
    att = att - corr

    m = att.max(axis=1, keepdims=True)
    e = np.exp(att - m)
    out = e / e.sum(axis=1, keepdims=True)
    return out.astype(np.float32)
